# revision 3
# baseline (speedup 1.0000x reference)
"""Trainium2 Bass kernel for nn_CenterDistLoss (segment_reduce).

Strategy (data-parallel over batch, 4 batches per core on 8 cores):
  Tiles are [128 partitions, 4096] = image row-block r across all 4 local
  batches (1024 columns each).
    labels = RNE-round(y_pr) * mask as fp16 (prep runs on GpSimd so the
             DVE spends all its time on the 27 is_equal passes)
    for l in 1..27:
      E_l = is_equal(labels, l) -> fp16        (DVE tensor_scalar, 4x mode)
      per (batch, col-half): one PE matmul [128, 512] against fp16 weights
      whose two live columns are (ones, 128*r + k)  -- global row index is
      fp16-exact (<= 1023 < 2048).  Labels are striped over the four 32-wide
      PE column groups via tile_position=(0, 32g), so matmuls of up to four
      consecutive labels run concurrently in the array.
  PSUM: one full [128, 512] bank per (batch, col-half); label l occupies
  rows 32*g + 2*idx (count) and +1 (row-index sum), g=(l-1)%4, idx=(l-1)//4.
  Host reduces the [4, 2, 128, 512] tables to centroids and the scalar
  loss (exact mirror of the reference).
"""

import numpy as np

try:
    import concourse.bass as bass
except ImportError:  # grading env may not have trn_rl_repo on sys.path
    import sys

    sys.path.insert(0, "/opt/trn_rl_repo")
    import concourse.bass as bass

import concourse.bacc as bacc
import concourse.mybir as mybir
from concourse.tile import TileContext
from concourse.bass_utils import run_bass_kernel_spmd
from contextlib import ExitStack

fp32 = mybir.dt.float32
bf16 = mybir.dt.bfloat16
fp16 = mybir.dt.float16

B, H, W = 32, 1024, 1024
N_CORES = 8
B_LOC = B // N_CORES  # 4 batches per core
P = 128
RB = H // P  # 8 row blocks
NL = 28  # label slots 0..27; only 1..27 computed
FW = B_LOC * W  # free width of a work tile
MAGIC = float(2**23)

L = 64  # reference label-table size


def _mean_dist_table():
    md = np.full(L, 14.0, dtype=np.float32)
    dists = {2: 18, 3: 18, 4: 18.5, 5: 19, 6: 19.5, 7: 20, 8: 20, 9: 20,
             10: 20.5, 11: 21, 12: 21.5, 13: 22, 14: 22.5, 15: 23, 16: 24.5,
             17: 24.5, 18: 26.5, 19: 28.5, 20: 29.5, 21: 33, 22: 33, 23: 33,
             24: 33, 25: 33, 26: 33}
    for k, v in dists.items():
        md[k] = v
    md[27:] = 30.0
    return md


MEAN_DIST = _mean_dist_table()


def _grp(l):
    return (l - 1) % 4


def _idx(l):
    return (l - 1) // 4


def build_weights() -> np.ndarray:
    """W[l-1, r, k, m] fp16 [27, 8, 128, 32]: within label l's column group,
    col 2*idx = 1.0 (count), col 2*idx+1 = 128*r + k (global row index)."""
    wts = np.zeros((NL - 1, RB, P, 32), np.float16)
    k = np.arange(P, dtype=np.float32)
    for l in range(1, NL):
        i = _idx(l)
        for r in range(RB):
            wts[l - 1, r, :, 2 * i] = 1.0
            wts[l - 1, r, :, 2 * i + 1] = 128.0 * r + k
    return wts


def build_nc() -> bass.Bass:
    nc = bacc.Bacc(trn_type="TRN2")
    y = nc.dram_tensor("y", [B_LOC, H, W], fp32, kind="ExternalInput")
    m = nc.dram_tensor("m", [B_LOC, H, W], fp32, kind="ExternalInput")
    wc = nc.dram_tensor("wc", [NL - 1, RB, P, 32], fp16, kind="ExternalInput")
    col_out = nc.dram_tensor("colfull", [B_LOC, 2, P, W // 2], fp32, kind="ExternalOutput")

    with TileContext(nc) as tc, ExitStack() as ctx:
        io = ctx.enter_context(tc.tile_pool(name="io", bufs=2))
        work = ctx.enter_context(tc.tile_pool(name="work", bufs=2))
        epool = ctx.enter_context(tc.tile_pool(name="epool", bufs=6))
        cpool = ctx.enter_context(tc.tile_pool(name="cpool", bufs=1))
        psum = ctx.enter_context(tc.tile_pool(name="psum", bufs=1, space="PSUM"))

        wts = cpool.tile([P, NL - 1, RB, 32], fp16, name="wts")
        nc.sync.dma_start(wts[:], wc.rearrange("l r k m -> k l r m"))

        # one full PSUM bank per (batch, col-half); 4 label groups stacked
        # on the partition axis inside each bank
        ps = [
            [psum.tile([P, W // 2], fp32, name=f"ps_{b}_{cb}") for cb in range(2)]
            for b in range(B_LOC)
        ]

        for r in range(RB):
            ytile = io.tile([P, FW], fp32, name="ytile", tag="ytile")
            mtile = io.tile([P, FW], fp32, name="mtile", tag="mtile")
            nc.sync.dma_start(
                ytile[:], y[:, r * P : (r + 1) * P, :].rearrange("b p w -> p b w")
            )
            nc.sync.dma_start(
                mtile[:], m[:, r * P : (r + 1) * P, :].rearrange("b p w -> p b w")
            )
            # label prep on GpSimd: RNE-round in fp32 (magic add/sub), then
            # mask multiply; masked-out pixels land on label 0, never compared
            ry = work.tile([P, FW], fp32, name="ry", tag="ry")
            nc.gpsimd.tensor_scalar(
                ry[:],
                ytile[:],
                MAGIC,
                MAGIC,
                mybir.AluOpType.add,
                mybir.AluOpType.subtract,
            )
            lab = work.tile([P, FW], fp16, name="lab", tag="lab")
            nc.gpsimd.tensor_tensor(lab[:], ry[:], mtile[:], mybir.AluOpType.mult)
            for l in range(1, NL):
                g, i = _grp(l), _idx(l)
                e = epool.tile([P, FW], fp16, name="e", tag="e")
                nc.vector.tensor_scalar(
                    e[:], lab[:], float(l), None, mybir.AluOpType.is_equal
                )
                last_in_grp = max(ll for ll in range(1, NL) if _grp(ll) == g)
                start = r == 0 and i == 0
                stop = r == RB - 1 and l == last_in_grp
                for b in range(B_LOC):
                    for cb in range(2):
                        nc.tensor.matmul(
                            ps[b][cb][32 * g : 32 * g + 32, :],
                            wts[:, l - 1, r, :],
                            e[:, b * W + cb * (W // 2) : b * W + (cb + 1) * (W // 2)],
                            start=start,
                            stop=stop,
                            tile_position=(0, 32 * g),
                        )
        for b in range(B_LOC):
            for cb in range(2):
                drain = work.tile([P, W // 2], fp32, name="drain", tag="drain")
                nc.vector.tensor_copy(drain[:], ps[b][cb][:, :])
                nc.sync.dma_start(col_out[b, cb], drain[:])
    nc.finalize()
    return nc


_NC = None


def _get_nc():
    global _NC
    if _NC is None:
        _NC = build_nc()
    return _NC


def finalize(colfulls):
    """Reduce per-core tables to the scalar loss (mirrors the reference)."""
    counts = np.zeros((B, L), np.float64)
    ysum = np.zeros((B, L), np.float64)
    xsum = np.zeros((B, L), np.float64)
    half = W // 2
    wr = np.arange(half, dtype=np.float64)
    for c in range(N_CORES):
        cf = colfulls[c].astype(np.float64)  # [B_LOC, 2, 128, 512]
        for bl in range(B_LOC):
            b = c * B_LOC + bl
            for l in range(1, NL):
                row = 32 * _grp(l) + 2 * _idx(l)
                cnt_rows = cf[bl, :, row, :]      # [2, 512]
                ysum_rows = cf[bl, :, row + 1, :]
                counts[b, l] = cnt_rows.sum()
                ysum[b, l] = ysum_rows.sum()
                xsum[b, l] = (cnt_rows[0] * wr).sum() + (cnt_rows[1] * (wr + half)).sum()
    safe = np.maximum(counts, 1.0)
    yc = ysum / safe
    xc = xsum / safe
    present = counts > 0
    present[:, 0] = False
    pair_ok = present[:, 1:] & present[:, :-1]
    dist = np.sqrt((xc[:, 1:] - xc[:, :-1]) ** 2 + (yc[:, 1:] - yc[:, :-1]) ** 2)
    loss = np.where(pair_ok, np.abs(dist - MEAN_DIST[1:][None, :]), 0.0).sum()
    return np.float32(loss)


_WC = None


def kernel(y_pr: np.ndarray, mask: np.ndarray, _trace=False, _trace_kwargs=None):
    global _WC
    y = np.ascontiguousarray(np.asarray(y_pr, dtype=np.float32).reshape(B, H, W))
    m = np.ascontiguousarray(np.asarray(mask, dtype=np.float32))
    if _WC is None:
        _WC = build_weights()
    nc = _get_nc()
    in_maps = [
        {
            "y": y[c * B_LOC : (c + 1) * B_LOC],
            "m": m[c * B_LOC : (c + 1) * B_LOC],
            "wc": _WC,
        }
        for c in range(N_CORES)
    ]
    kw = {}
    if _trace:
        kw["trace"] = True
        kw.update(_trace_kwargs or {})
    res = run_bass_kernel_spmd(nc, in_maps, core_ids=list(range(N_CORES)), **kw)
    loss = finalize([r["colfull"] for r in res.results])
    if _trace:
        return loss, res
    return loss


# revision 4
# speedup vs baseline: 2.3548x; 2.3548x over previous
"""Trainium2 Bass kernel for nn_CenterDistLoss (segment_reduce).

Strategy (data-parallel over batch, 4 batches per core on 8 cores):
  Tiles are [128 partitions, 4096] = image row-block r across all 4 local
  batches (1024 columns each).
    labels = RNE-round(y_pr) * mask as fp16 (prep runs on GpSimd so the
             DVE spends all its time on the 27 is_equal passes)
    for l in 1..27:
      E_l = is_equal(labels, l) -> fp16        (DVE tensor_scalar, 4x mode)
      per (batch, col-half): one PE matmul [128, 512] against fp16 weights
      whose two live columns are (ones, 128*r + k)  -- global row index is
      fp16-exact (<= 1023 < 2048).  Labels are striped over the four 32-wide
      PE column groups via tile_position=(0, 32g), so matmuls of up to four
      consecutive labels run concurrently in the array.
  PSUM: one full [128, 512] bank per (batch, col-half); label l occupies
  rows 32*g + 2*idx (count) and +1 (row-index sum), g=(l-1)%4, idx=(l-1)//4.
  Host reduces the [4, 2, 128, 512] tables to centroids and the scalar
  loss (exact mirror of the reference).
"""

import numpy as np

try:
    import concourse.bass as bass
except ImportError:  # grading env may not have trn_rl_repo on sys.path
    import sys

    sys.path.insert(0, "/opt/trn_rl_repo")
    import concourse.bass as bass

import concourse.bacc as bacc
import concourse.mybir as mybir
from concourse.tile import TileContext
from concourse.bass_utils import run_bass_kernel_spmd
from contextlib import ExitStack

fp32 = mybir.dt.float32
bf16 = mybir.dt.bfloat16
fp16 = mybir.dt.float16

B, H, W = 32, 1024, 1024
N_CORES = 8
B_LOC = B // N_CORES  # 4 batches per core
P = 128
RB = H // P  # 8 row blocks
NL = 28  # label slots 0..27; only 1..27 computed
FW = B_LOC * W  # free width of a work tile
MAGIC = float(2**23)

L = 64  # reference label-table size


def _mean_dist_table():
    md = np.full(L, 14.0, dtype=np.float32)
    dists = {2: 18, 3: 18, 4: 18.5, 5: 19, 6: 19.5, 7: 20, 8: 20, 9: 20,
             10: 20.5, 11: 21, 12: 21.5, 13: 22, 14: 22.5, 15: 23, 16: 24.5,
             17: 24.5, 18: 26.5, 19: 28.5, 20: 29.5, 21: 33, 22: 33, 23: 33,
             24: 33, 25: 33, 26: 33}
    for k, v in dists.items():
        md[k] = v
    md[27:] = 30.0
    return md


MEAN_DIST = _mean_dist_table()


def _grp(l):
    return (l - 1) % 4


def _idx(l):
    return (l - 1) // 4


def build_weights() -> np.ndarray:
    """W[l-1, r, k, m] bf16 [27, 8, 128, 32]: within label l's column group,
    col 3*idx = 1.0 (count), 3*idx+1 = k (local row), 3*idx+2 = 128*r."""
    import ml_dtypes

    wts = np.zeros((NL - 1, RB, P, 32), ml_dtypes.bfloat16)
    k = np.arange(P, dtype=np.float32)
    for l in range(1, NL):
        i = _idx(l)
        for r in range(RB):
            wts[l - 1, r, :, 3 * i] = 1.0
            wts[l - 1, r, :, 3 * i + 1] = k
            wts[l - 1, r, :, 3 * i + 2] = float(128 * r)
    return wts


def build_nc() -> bass.Bass:
    nc = bacc.Bacc(trn_type="TRN2")
    y = nc.dram_tensor("y", [B_LOC, H, W], fp32, kind="ExternalInput")
    m = nc.dram_tensor("m", [B_LOC, H, W], fp32, kind="ExternalInput")
    wc = nc.dram_tensor("wc", [NL - 1, RB, P, 32], bf16, kind="ExternalInput")
    col_out = nc.dram_tensor("colfull", [B_LOC, 2, P, W // 2], fp32, kind="ExternalOutput")

    with TileContext(nc) as tc, ExitStack() as ctx:
        io = ctx.enter_context(tc.tile_pool(name="io", bufs=2))
        work = ctx.enter_context(tc.tile_pool(name="work", bufs=2))
        epool = ctx.enter_context(tc.tile_pool(name="epool", bufs=6))
        cpool = ctx.enter_context(tc.tile_pool(name="cpool", bufs=1))
        psum = ctx.enter_context(tc.tile_pool(name="psum", bufs=1, space="PSUM"))

        wts = cpool.tile([P, NL - 1, RB, 32], bf16, name="wts")
        nc.sync.dma_start(wts[:], wc.rearrange("l r k m -> k l r m"))

        # one full PSUM bank per (batch, col-half); 4 label groups stacked
        # on the partition axis inside each bank
        ps = [
            [psum.tile([P, W // 2], fp32, name=f"ps_{b}_{cb}") for cb in range(2)]
            for b in range(B_LOC)
        ]

        for r in range(RB):
            ytile = io.tile([P, FW], fp32, name="ytile", tag="ytile")
            mtile = io.tile([P, FW], fp32, name="mtile", tag="mtile")
            nc.sync.dma_start(
                ytile[:], y[:, r * P : (r + 1) * P, :].rearrange("b p w -> p b w")
            )
            nc.sync.dma_start(
                mtile[:], m[:, r * P : (r + 1) * P, :].rearrange("b p w -> p b w")
            )
            # label prep on GpSimd: RNE-round in fp32 (magic add/sub), then
            # mask multiply; masked-out pixels land on label 0, never compared
            ry = work.tile([P, FW], fp32, name="ry", tag="ry")
            nc.vector.tensor_scalar(
                ry[:],
                ytile[:],
                MAGIC,
                MAGIC,
                mybir.AluOpType.add,
                mybir.AluOpType.subtract,
            )
            lab = work.tile([P, FW], bf16, name="lab", tag="lab")
            nc.vector.tensor_tensor(lab[:], ry[:], mtile[:], mybir.AluOpType.mult)
            for l in range(1, NL):
                g, i = _grp(l), _idx(l)
                e = epool.tile([P, FW], bf16, name="e", tag="e")
                nc.vector.tensor_scalar(
                    e[:], lab[:], float(l), None, mybir.AluOpType.is_equal
                )
                last_in_grp = max(ll for ll in range(1, NL) if _grp(ll) == g)
                start = r == 0 and i == 0
                stop = r == RB - 1 and l == last_in_grp
                for b in range(B_LOC):
                    for cb in range(2):
                        nc.tensor.matmul(
                            ps[b][cb][32 * g : 32 * g + 32, :],
                            wts[:, l - 1, r, :],
                            e[:, b * W + cb * (W // 2) : b * W + (cb + 1) * (W // 2)],
                            start=start,
                            stop=stop,
                            tile_position=(0, 32 * g),
                        )
        for b in range(B_LOC):
            for cb in range(2):
                drain = work.tile([P, W // 2], fp32, name="drain", tag="drain")
                nc.vector.tensor_copy(drain[:], ps[b][cb][:, :])
                nc.sync.dma_start(col_out[b, cb], drain[:])
    nc.finalize()
    return nc


_NC = None


def _get_nc():
    global _NC
    if _NC is None:
        _NC = build_nc()
    return _NC


def finalize(colfulls):
    """Reduce per-core tables to the scalar loss (mirrors the reference)."""
    counts = np.zeros((B, L), np.float64)
    ysum = np.zeros((B, L), np.float64)
    xsum = np.zeros((B, L), np.float64)
    half = W // 2
    wr = np.arange(half, dtype=np.float64)
    for c in range(N_CORES):
        cf = colfulls[c].astype(np.float64)  # [B_LOC, 2, 128, 512]
        for bl in range(B_LOC):
            b = c * B_LOC + bl
            for l in range(1, NL):
                row = 32 * _grp(l) + 3 * _idx(l)
                cnt_rows = cf[bl, :, row, :]      # [2, 512]
                ysum_rows = cf[bl, :, row + 1, :] + cf[bl, :, row + 2, :]
                counts[b, l] = cnt_rows.sum()
                ysum[b, l] = ysum_rows.sum()
                xsum[b, l] = (cnt_rows[0] * wr).sum() + (cnt_rows[1] * (wr + half)).sum()
    safe = np.maximum(counts, 1.0)
    yc = ysum / safe
    xc = xsum / safe
    present = counts > 0
    present[:, 0] = False
    pair_ok = present[:, 1:] & present[:, :-1]
    dist = np.sqrt((xc[:, 1:] - xc[:, :-1]) ** 2 + (yc[:, 1:] - yc[:, :-1]) ** 2)
    loss = np.where(pair_ok, np.abs(dist - MEAN_DIST[1:][None, :]), 0.0).sum()
    return np.float32(loss)


_WC = None


def kernel(y_pr: np.ndarray, mask: np.ndarray, _trace=False, _trace_kwargs=None):
    global _WC
    y = np.ascontiguousarray(np.asarray(y_pr, dtype=np.float32).reshape(B, H, W))
    m = np.ascontiguousarray(np.asarray(mask, dtype=np.float32))
    if _WC is None:
        _WC = build_weights()
    nc = _get_nc()
    in_maps = [
        {
            "y": y[c * B_LOC : (c + 1) * B_LOC],
            "m": m[c * B_LOC : (c + 1) * B_LOC],
            "wc": _WC,
        }
        for c in range(N_CORES)
    ]
    kw = {}
    if _trace:
        kw["trace"] = True
        kw.update(_trace_kwargs or {})
    res = run_bass_kernel_spmd(nc, in_maps, core_ids=list(range(N_CORES)), **kw)
    loss = finalize([r["colfull"] for r in res.results])
    if _trace:
        return loss, res
    return loss


# revision 5
# speedup vs baseline: 2.4500x; 1.0404x over previous
"""Trainium2 Bass kernel for nn_CenterDistLoss (segment_reduce).

Strategy (data-parallel over batch, 4 batches per core on 8 cores):
  Tiles are [128 partitions, 4096] = image row-block r across all 4 local
  batches (1024 columns each).
    labels = RNE-round(y_pr) * mask as bf16 (round on DVE fp32-in/bf16-out;
             mask cast fp32->bf16 on the Scalar engine via exact Relu)
    for l in 1..27:
      E_l = is_equal(labels, l) -> fp16        (DVE tensor_scalar, 4x mode)
      per (batch, col-half): one PE matmul [128, 512] against fp16 weights
      whose two live columns are (ones, 128*r + k)  -- global row index is
      fp16-exact (<= 1023 < 2048).  Labels are striped over the four 32-wide
      PE column groups via tile_position=(0, 32g), so matmuls of up to four
      consecutive labels run concurrently in the array.
  PSUM: one full [128, 512] bank per (batch, col-half); label l occupies
  rows 32*g + 2*idx (count) and +1 (row-index sum), g=(l-1)%4, idx=(l-1)//4.
  Host reduces the [4, 2, 128, 512] tables to centroids and the scalar
  loss (exact mirror of the reference).
"""

import numpy as np

try:
    import concourse.bass as bass
except ImportError:  # grading env may not have trn_rl_repo on sys.path
    import sys

    sys.path.insert(0, "/opt/trn_rl_repo")
    import concourse.bass as bass

import concourse.bacc as bacc
import concourse.mybir as mybir
from concourse.tile import TileContext
from concourse.bass_utils import run_bass_kernel_spmd
from contextlib import ExitStack

fp32 = mybir.dt.float32
bf16 = mybir.dt.bfloat16
fp16 = mybir.dt.float16

B, H, W = 32, 1024, 1024
N_CORES = 8
B_LOC = B // N_CORES  # 4 batches per core
P = 128
RB = H // P  # 8 row blocks
NL = 28  # label slots 0..27; only 1..27 computed
FW = B_LOC * W  # free width of a work tile
MAGIC = float(2**23)

L = 64  # reference label-table size


def _mean_dist_table():
    md = np.full(L, 14.0, dtype=np.float32)
    dists = {2: 18, 3: 18, 4: 18.5, 5: 19, 6: 19.5, 7: 20, 8: 20, 9: 20,
             10: 20.5, 11: 21, 12: 21.5, 13: 22, 14: 22.5, 15: 23, 16: 24.5,
             17: 24.5, 18: 26.5, 19: 28.5, 20: 29.5, 21: 33, 22: 33, 23: 33,
             24: 33, 25: 33, 26: 33}
    for k, v in dists.items():
        md[k] = v
    md[27:] = 30.0
    return md


MEAN_DIST = _mean_dist_table()


def _grp(l):
    return (l - 1) % 4


def _idx(l):
    return (l - 1) // 4


def build_weights() -> np.ndarray:
    """W[l-1, r, k, m] bf16 [27, 8, 128, 32]: within label l's column group,
    col 3*idx = 1.0 (count), 3*idx+1 = k (local row), 3*idx+2 = 128*r."""
    import ml_dtypes

    wts = np.zeros((NL - 1, RB, P, 32), ml_dtypes.bfloat16)
    k = np.arange(P, dtype=np.float32)
    for l in range(1, NL):
        i = _idx(l)
        for r in range(RB):
            wts[l - 1, r, :, 3 * i] = 1.0
            wts[l - 1, r, :, 3 * i + 1] = k
            wts[l - 1, r, :, 3 * i + 2] = float(128 * r)
    return wts


def build_nc() -> bass.Bass:
    nc = bacc.Bacc(trn_type="TRN2")
    y = nc.dram_tensor("y", [B_LOC, H, W], fp32, kind="ExternalInput")
    m = nc.dram_tensor("m", [B_LOC, H, W], fp32, kind="ExternalInput")
    wc = nc.dram_tensor("wc", [NL - 1, RB, P, 32], bf16, kind="ExternalInput")
    col_out = nc.dram_tensor("colfull", [B_LOC, 2, P, W // 2], fp32, kind="ExternalOutput")

    with TileContext(nc) as tc, ExitStack() as ctx:
        io = ctx.enter_context(tc.tile_pool(name="io", bufs=2))
        work = ctx.enter_context(tc.tile_pool(name="work", bufs=2))
        epool = ctx.enter_context(tc.tile_pool(name="epool", bufs=6))
        cpool = ctx.enter_context(tc.tile_pool(name="cpool", bufs=1))
        psum = ctx.enter_context(tc.tile_pool(name="psum", bufs=1, space="PSUM"))

        wts = cpool.tile([P, NL - 1, RB, 32], bf16, name="wts")
        nc.sync.dma_start(wts[:], wc.rearrange("l r k m -> k l r m"))

        # one full PSUM bank per (batch, col-half); 4 label groups stacked
        # on the partition axis inside each bank
        ps = [
            [psum.tile([P, W // 2], fp32, name=f"ps_{b}_{cb}") for cb in range(2)]
            for b in range(B_LOC)
        ]

        for r in range(RB):
            ytile = io.tile([P, FW], fp32, name="ytile", tag="ytile")
            mtile = io.tile([P, FW], fp32, name="mtile", tag="mtile")
            nc.sync.dma_start(
                ytile[:], y[:, r * P : (r + 1) * P, :].rearrange("b p w -> p b w")
            )
            nc.sync.dma_start(
                mtile[:], m[:, r * P : (r + 1) * P, :].rearrange("b p w -> p b w")
            )
            # label prep on GpSimd: RNE-round in fp32 (magic add/sub), then
            # mask multiply; masked-out pixels land on label 0, never compared
            ry = work.tile([P, FW], bf16, name="ry", tag="ry")
            nc.vector.tensor_scalar(
                ry[:],
                ytile[:],
                MAGIC,
                MAGIC,
                mybir.AluOpType.add,
                mybir.AluOpType.subtract,
            )
            mb = work.tile([P, FW], bf16, name="mb", tag="mb")
            nc.scalar.activation(mb[:], mtile[:], mybir.ActivationFunctionType.Relu)
            lab = work.tile([P, FW], bf16, name="lab", tag="lab")
            nc.vector.tensor_tensor(lab[:], ry[:], mb[:], mybir.AluOpType.mult)
            for l in range(1, NL):
                g, i = _grp(l), _idx(l)
                e = epool.tile([P, FW], bf16, name="e", tag="e")
                nc.vector.tensor_scalar(
                    e[:], lab[:], float(l), None, mybir.AluOpType.is_equal
                )
                last_in_grp = max(ll for ll in range(1, NL) if _grp(ll) == g)
                start = r == 0 and i == 0
                stop = r == RB - 1 and l == last_in_grp
                for b in range(B_LOC):
                    for cb in range(2):
                        nc.tensor.matmul(
                            ps[b][cb][32 * g : 32 * g + 32, :],
                            wts[:, l - 1, r, :],
                            e[:, b * W + cb * (W // 2) : b * W + (cb + 1) * (W // 2)],
                            start=start,
                            stop=stop,
                            tile_position=(0, 32 * g),
                        )
        for b in range(B_LOC):
            for cb in range(2):
                drain = work.tile([P, W // 2], fp32, name="drain", tag="drain")
                nc.vector.tensor_copy(drain[:], ps[b][cb][:, :])
                nc.sync.dma_start(col_out[b, cb], drain[:])
    nc.finalize()
    return nc


_NC = None


def _get_nc():
    global _NC
    if _NC is None:
        _NC = build_nc()
    return _NC


def finalize(colfulls):
    """Reduce per-core tables to the scalar loss (mirrors the reference)."""
    counts = np.zeros((B, L), np.float64)
    ysum = np.zeros((B, L), np.float64)
    xsum = np.zeros((B, L), np.float64)
    half = W // 2
    wr = np.arange(half, dtype=np.float64)
    for c in range(N_CORES):
        cf = colfulls[c].astype(np.float64)  # [B_LOC, 2, 128, 512]
        for bl in range(B_LOC):
            b = c * B_LOC + bl
            for l in range(1, NL):
                row = 32 * _grp(l) + 3 * _idx(l)
                cnt_rows = cf[bl, :, row, :]      # [2, 512]
                ysum_rows = cf[bl, :, row + 1, :] + cf[bl, :, row + 2, :]
                counts[b, l] = cnt_rows.sum()
                ysum[b, l] = ysum_rows.sum()
                xsum[b, l] = (cnt_rows[0] * wr).sum() + (cnt_rows[1] * (wr + half)).sum()
    safe = np.maximum(counts, 1.0)
    yc = ysum / safe
    xc = xsum / safe
    present = counts > 0
    present[:, 0] = False
    pair_ok = present[:, 1:] & present[:, :-1]
    dist = np.sqrt((xc[:, 1:] - xc[:, :-1]) ** 2 + (yc[:, 1:] - yc[:, :-1]) ** 2)
    loss = np.where(pair_ok, np.abs(dist - MEAN_DIST[1:][None, :]), 0.0).sum()
    return np.float32(loss)


_WC = None


def kernel(y_pr: np.ndarray, mask: np.ndarray, _trace=False, _trace_kwargs=None):
    global _WC
    y = np.ascontiguousarray(np.asarray(y_pr, dtype=np.float32).reshape(B, H, W))
    m = np.ascontiguousarray(np.asarray(mask, dtype=np.float32))
    if _WC is None:
        _WC = build_weights()
    nc = _get_nc()
    in_maps = [
        {
            "y": y[c * B_LOC : (c + 1) * B_LOC],
            "m": m[c * B_LOC : (c + 1) * B_LOC],
            "wc": _WC,
        }
        for c in range(N_CORES)
    ]
    kw = {}
    if _trace:
        kw["trace"] = True
        kw.update(_trace_kwargs or {})
    res = run_bass_kernel_spmd(nc, in_maps, core_ids=list(range(N_CORES)), **kw)
    loss = finalize([r["colfull"] for r in res.results])
    if _trace:
        return loss, res
    return loss
